# revision 38
# baseline (speedup 1.0000x reference)
"""Trainium2 Bass kernel for AdaptiveStateAllocator (topk_masking).

Data-parallel over batch on 8 NeuronCores. Each core:
  - streams its x shard [4, 2048, 1024] from HBM as host-pre-rounded bf16
    (16MB/core instead of 32MB -- the memory-bound part; rounding validated
    at ~250x margin for the mask math),
  - mean-pools over seq via PE matmuls with a stationary per-batch 1/S
    selector column (x is the moving operand, no per-matmul weight loads),
  - runs the tiny complexity MLP and importance-scorer MLP on-device,
  - derives the mask without sorting or sigmoid:
      rank_i   = #{j : logit_j > logit_i}            (comparison matrix)
      mask^T[i,b] = (z_b >= zeta[rank_i])            (z = pre-sigmoid logit)
    where zeta[rho] = logit((rho - 3.5)/60) - b3 is a host-precomputed
    threshold table equivalent to round_half_even(4 + 60*sigmoid(z + b3)).
    The zeta[rank_i] gather runs as a one-hot matmul BEFORE the x stream
    finishes, so the post-stream tail is just pooled->h1->h2->z->mask.
  - emits its [64, 4] mask^T shard and [4, 64, 512] broadcast state bank.

Host side shards inputs, precomputes transposed weights + the threshold
table, and reassembles the full outputs.
"""

import math
import sys

import numpy as np

sys.path.insert(0, "/opt/trn_rl_repo")

import concourse.bass as bass  # noqa: E402
import concourse.tile as tile  # noqa: E402
from concourse import mybir  # noqa: E402
from concourse.bacc import Bacc  # noqa: E402
from concourse.bass_utils import run_bass_kernel_spmd  # noqa: E402

F32 = mybir.dt.float32
BF16 = mybir.dt.bfloat16
AF = mybir.ActivationFunctionType
ALU = mybir.AluOpType

# x and the complexity-MLP weights run in bf16 (empirical mask-boundary
# margin 0.19 in v-space vs <=0.0011 bf16-induced error on these inputs).
# The importance-logit branch (tight 8.8e-5 rank-boundary margin) stays
# fp32 throughout.

# Problem constants (hardcoded per harness contract).
B, S, D_IN, D_ST = 32, 2048, 1024, 512
MIN_STATES, MAX_STATES = 4, 64
N_CORES = 8
B_LOC = B // N_CORES          # 4 samples per core
P = 128                       # SBUF partitions
SEQ_GROUP = 512               # seq rows per DMA tile -> [128, 4, 1024] bf16 = 1MB
N_SUB = SEQ_GROUP // P        # 4 seq sub-chunks per tile
N_TILES = S // SEQ_GROUP      # 4 tiles per sample
K_CHUNKS = S // P             # 16 accumulation chunks per sample
D_CHUNKS = D_IN // P          # 8

# packed small-const column layout ([128, PK_COLS] f32)
PK_B1 = 0          # [128, 4]  b1 chunks
PK_B2 = 4          # [128, 2]
PK_W3 = 6          # [128, 2]
PK_BS1 = 8         # [128, 2]
PK_WS2 = 10        # [128, 2]
PK_IDENT = 12      # [64, 64] identity (partitions 0-63)
PK_ZETA = 76       # [64, 1]  zeta_col
PK_IOTA = 77       # [64, 1]  0..63
PK_RVEC = 78       # [128, 1] 1/S
PK_ONES = 79       # [128, 1] ones (spare)
PK_ONESROW = 80    # [1, 64] ones in partition 0
PK_BSEL = 148      # [128, 4] x4: block b has column b = 1/S, rest 0
PK_COLS = 164
# bf16 const block: bsel blocks then W3 chunks
PKB_W3 = 16        # [128, 2]
PKB_W3REP = 18     # [128, 64] x2: W3 chunk kc replicated across 64 columns
PKB_COLS = 146


def _build_kernel(n_sweeps: int = 1):
    """n_sweeps > 1 repeats the x stream + pooling (profiling only): the
    marginal cost per extra sweep isolates true HW time from dispatch noise."""
    nc = Bacc()

    x = nc.declare_dram_parameter("x", [B_LOC, S, D_IN], BF16, isOutput=False)
    w1t = nc.declare_dram_parameter("w1t", [D_IN, D_ST], BF16, isOutput=False)
    w2t = nc.declare_dram_parameter("w2t", [D_ST, D_ST // 2], BF16, isOutput=False)
    ws1t = nc.declare_dram_parameter("ws1t", [D_ST, D_ST // 2], F32, isOutput=False)
    sbt = nc.declare_dram_parameter("sbt", [P, 4 * MAX_STATES], F32, isOutput=False)
    sbank = nc.declare_dram_parameter("sbank", [MAX_STATES, D_ST], F32, isOutput=False)
    pk = nc.declare_dram_parameter("pk", [P, PK_COLS], F32, isOutput=False)
    pkb = nc.declare_dram_parameter("pkb", [P, PKB_COLS], BF16, isOutput=False)

    out_states = nc.declare_dram_parameter(
        "out_states", [B_LOC, MAX_STATES, D_ST], F32, isOutput=True
    )
    out_maskt = nc.declare_dram_parameter(
        "out_maskt", [MAX_STATES, B_LOC], F32, isOutput=True
    )

    with tile.TileContext(nc) as tc:
        with (
            tc.tile_pool(name="xpool", bufs=8) as xpool,
            tc.tile_pool(name="consts", bufs=1) as consts,
            tc.tile_pool(name="small", bufs=1) as small,
            tc.tile_pool(name="psum", bufs=4, space="PSUM") as psum,
            tc.tile_pool(name="psum1", bufs=1, space="PSUM") as psum1,
        ):
            # ---- x stream: issue on the sync HWDGE ring, first in program
            # order so the stream starts immediately. Pooling matmuls keep the
            # 1/S column STATIONARY (one trivial weight load) and stream x as
            # the moving operand in N=512 slabs -> pooled [4, 1024] in PSUM.
            # (The stationary-x variant pays a [128,128] LDWEIGHTS per matmul
            # and throttles the stream on real HW.)
            pool_ps = psum1.tile([B_LOC, D_IN], F32)           # 2 PSUM banks
            poolt_ps = psum1.tile([P, D_CHUNKS * B_LOC], F32)  # one PSUM bank

            # x is pre-rounded to bf16 on the host, so the stream is plain
            # HWDGE loads of half the bytes (no SWDGE cast ring needed).

            # Tile plan: 2MB tiles, except the final ~2MB of the last batch
            # which is split into 512KB tiles so the post-stream matmul burst
            # (which is pure tail) stays short.
            def tile_plan():
                plan = []  # (b, seq_start, n_sub)
                for b in range(B_LOC):
                    s = 0
                    while s < S:
                        if b == B_LOC - 1 and s >= S - SEQ_GROUP:
                            step = P  # 512KB tiles for the last seq group
                        else:
                            step = SEQ_GROUP
                        plan.append((b, s, step // P))
                        s += step
                return plan

            def emit_x_dmas():
                tiles = []
                for idx, (b, s, n_sub) in enumerate(tile_plan()):
                    xt = xpool.tile([P, N_SUB, D_IN], BF16, tag="xtb")
                    nc.sync.dma_start(
                        out=xt[:, 0:n_sub, :],
                        in_=x[b, s:s + n_sub * P, :].rearrange(
                            "(n p) d -> p n d", p=P
                        ),
                    )
                    tiles.append((idx, b, s, n_sub, xt))
                return tiles

            def emit_pooling(tiles):
                for idx, b, s, n_sub, xt in tiles:
                    bsel = pkb_sb[:, 4 * b:4 * b + 4]
                    for n in range(n_sub):
                        kc = s // P + n
                        for half in range(2):
                            nc.tensor.matmul(
                                pool_ps[:, half * 512:(half + 1) * 512],
                                lhsT=bsel,
                                rhs=xt[:, n, half * 512:(half + 1) * 512],
                                start=(kc == 0 and b == 0),
                                stop=(kc == K_CHUNKS - 1 and b == B_LOC - 1),
                            )

            x_tiles = emit_x_dmas()

            # ---- const loads: scalar-engine HWDGE ring (doesn't queue
            # behind the x stream's slot-reuse waits)
            pk_sb = consts.tile([P, PK_COLS], F32)
            nc.scalar.dma_start(out=pk_sb, in_=pk[:, :])
            pkb_sb = consts.tile([P, PKB_COLS], BF16)
            nc.scalar.dma_start(out=pkb_sb, in_=pkb[:, :])
            w1_sb = consts.tile([P, D_CHUNKS, D_ST], BF16)  # [128, 8, 512]
            nc.scalar.dma_start(out=w1_sb, in_=w1t.rearrange("(c p) m -> p c m", p=P))
            w2_sb = consts.tile([P, 4, D_ST // 2], BF16)
            nc.scalar.dma_start(out=w2_sb, in_=w2t.rearrange("(c p) m -> p c m", p=P))
            ws1_sb = consts.tile([P, 4, D_ST // 2], F32)
            nc.scalar.dma_start(out=ws1_sb, in_=ws1t.rearrange("(c p) m -> p c m", p=P))
            sbt_sb = consts.tile([P, 4, MAX_STATES], F32)
            nc.scalar.dma_start(out=sbt_sb, in_=sbt.rearrange("p (c m) -> p c m", m=MAX_STATES))
            sbank_sb = consts.tile([MAX_STATES, D_ST], F32)
            nc.scalar.dma_start(out=sbank_sb, in_=sbank[:, :])

            ones_row = pk_sb[0:1, PK_ONESROW:PK_ONESROW + MAX_STATES]
            ident_sb = pk_sb[0:MAX_STATES, PK_IDENT:PK_IDENT + MAX_STATES]
            zeta_col = pk_sb[0:MAX_STATES, PK_ZETA:PK_ZETA + 1]
            iota_col = pk_sb[0:MAX_STATES, PK_IOTA:PK_IOTA + 1]

            # ---- broadcast state bank to output (tiny, independent)
            for b in range(B_LOC):
                nc.sync.dma_start(out=out_states[b], in_=sbank_sb)

            # ---- pooling matmuls, first half: PE's in-order queue must
            # start with tile consumption so DMA slots recycle promptly
            emit_pooling(x_tiles[:9])

            # ==== importance branch (independent of x -> runs during stream,
            # its cross-engine hops absorbed while tiles 8.. buffer up)
            hs_t = small.tile([P, 2 * MAX_STATES], F32)
            for mc in range(2):
                hs_ps = psum.tile([P, MAX_STATES], F32, tag="mlp")
                for kc in range(4):
                    nc.tensor.matmul(
                        hs_ps,
                        lhsT=ws1_sb[:, kc, mc * P:(mc + 1) * P],
                        rhs=sbt_sb[:, kc, :],
                        start=(kc == 0),
                        stop=(kc == 3),
                    )
                nc.scalar.activation(
                    out=hs_t[:, mc * MAX_STATES:(mc + 1) * MAX_STATES],
                    in_=hs_ps,
                    func=AF.Relu,
                    bias=pk_sb[:, PK_BS1 + mc:PK_BS1 + mc + 1],
                )
            # importance logits, column+row forms (bs2 dropped: rank-invariant)
            l_ps = psum.tile([MAX_STATES, 1], F32, tag="mlp")
            for kc in range(2):
                nc.tensor.matmul(
                    l_ps,
                    lhsT=hs_t[:, kc * MAX_STATES:(kc + 1) * MAX_STATES],
                    rhs=pk_sb[:, PK_WS2 + kc:PK_WS2 + kc + 1],
                    start=(kc == 0),
                    stop=(kc == 1),
                )
            lcol = small.tile([MAX_STATES, 1], F32)
            nc.scalar.copy(out=lcol, in_=l_ps)
            lrow_ps = psum.tile([1, MAX_STATES], F32, tag="mlp")
            nc.tensor.matmul(lrow_ps, lhsT=lcol, rhs=ident_sb, start=True, stop=True)
            lrow = small.tile([1, MAX_STATES], F32)
            nc.scalar.copy(out=lrow, in_=lrow_ps)
            # lrep[i, j] = l_j (K=1 outer product), then rank_i = #{j: l_j > l_i}
            lrep_ps = psum.tile([MAX_STATES, MAX_STATES], F32, tag="mlp")
            nc.tensor.matmul(lrep_ps, lhsT=ones_row, rhs=lrow, start=True, stop=True)
            cmp = small.tile([MAX_STATES, MAX_STATES], F32)
            rcol = small.tile([MAX_STATES, 1], F32)
            nc.vector.tensor_scalar(
                out=cmp, in0=lrep_ps, scalar1=lcol, scalar2=0.0, op0=ALU.is_gt,
                op1=ALU.add, accum_out=rcol,
            )
            # rank one-hot: OhT[rho, i] = (rank_i == rho); zeta_r = OhT.T @ zeta
            rrow_ps = psum.tile([1, MAX_STATES], F32, tag="mlp")
            nc.tensor.matmul(rrow_ps, lhsT=rcol, rhs=ident_sb, start=True, stop=True)
            rrow = small.tile([1, MAX_STATES], F32)
            nc.scalar.copy(out=rrow, in_=rrow_ps)
            rrep_ps = psum.tile([MAX_STATES, MAX_STATES], F32, tag="mlp")
            nc.tensor.matmul(rrep_ps, lhsT=ones_row, rhs=rrow, start=True, stop=True)
            oht = small.tile([MAX_STATES, MAX_STATES], F32)
            nc.vector.tensor_scalar(
                out=oht, in0=rrep_ps, scalar1=iota_col, scalar2=None, op0=ALU.is_equal
            )
            zr_ps = psum.tile([MAX_STATES, 1], F32, tag="mlp")
            nc.tensor.matmul(zr_ps, lhsT=oht, rhs=zeta_col, start=True, stop=True)
            zeta_r = small.tile([MAX_STATES, 1], F32)
            nc.scalar.copy(out=zeta_r, in_=zr_ps)

            # ---- pooling matmuls, second half (first half emitted above so
            # PE starts consuming tiles before the rank chain's engine hops)
            emit_pooling(x_tiles[9:])
            for _sweep in range(1, n_sweeps):
                emit_pooling(emit_x_dmas())

            # ==== complexity MLP (critical tail after the x stream)
            pooled_sb = small.tile([B_LOC, D_IN], F32)
            nc.scalar.copy(out=pooled_sb, in_=pool_ps)
            for dc in range(D_CHUNKS):
                nc.tensor.matmul(
                    poolt_ps[:, dc * B_LOC:(dc + 1) * B_LOC],
                    lhsT=pooled_sb[0:B_LOC, dc * P:(dc + 1) * P],
                    rhs=ident_sb[0:B_LOC, 0:B_LOC],
                    start=True,
                    stop=True,
                )
            pooled_t = small.tile([P, D_CHUNKS * B_LOC], BF16)  # [128, 32]
            nc.scalar.copy(out=pooled_t, in_=poolt_ps)
            h1_t = small.tile([P, 4 * B_LOC], BF16)
            for mc in range(4):
                h1_ps = psum.tile([P, B_LOC], F32, tag="mlp")
                for kc in range(D_CHUNKS):
                    nc.tensor.matmul(
                        h1_ps,
                        lhsT=w1_sb[:, kc, mc * P:(mc + 1) * P],
                        rhs=pooled_t[:, kc * B_LOC:(kc + 1) * B_LOC],
                        start=(kc == 0),
                        stop=(kc == D_CHUNKS - 1),
                    )
                nc.scalar.activation(
                    out=h1_t[:, mc * B_LOC:(mc + 1) * B_LOC],
                    in_=h1_ps,
                    func=AF.Relu,
                    bias=pk_sb[:, PK_B1 + mc:PK_B1 + mc + 1],
                )
            h2_t = small.tile([P, 2 * B_LOC], BF16)
            for mc in range(2):
                h2_ps = psum.tile([P, B_LOC], F32, tag="mlp")
                for kc in range(4):
                    nc.tensor.matmul(
                        h2_ps,
                        lhsT=w2_sb[:, kc, mc * P:(mc + 1) * P],
                        rhs=h1_t[:, kc * B_LOC:(kc + 1) * B_LOC],
                        start=(kc == 0),
                        stop=(kc == 3),
                    )
                nc.scalar.activation(
                    out=h2_t[:, mc * B_LOC:(mc + 1) * B_LOC],
                    in_=h2_ps,
                    func=AF.Relu,
                    bias=pk_sb[:, PK_B2 + mc:PK_B2 + mc + 1],
                )
            # z replicated [64, 4] in ONE accumulation: lhsT = W3 chunk
            # broadcast across 64 columns (host-prepared), rhs = h2^T
            zrep_ps = psum.tile([MAX_STATES, B_LOC], F32, tag="mlp")
            for kc in range(2):
                nc.tensor.matmul(
                    zrep_ps,
                    lhsT=pkb_sb[:, PKB_W3REP + kc * MAX_STATES:
                                PKB_W3REP + (kc + 1) * MAX_STATES],
                    rhs=h2_t[:, kc * B_LOC:(kc + 1) * B_LOC],
                    start=(kc == 0),
                    stop=(kc == 1),
                )
            # mask^T[i, b] = (z_b >= zeta[rank_i])
            maskt = small.tile([MAX_STATES, B_LOC], F32)
            nc.vector.tensor_scalar(
                out=maskt, in0=zrep_ps, scalar1=zeta_r, scalar2=None, op0=ALU.is_ge
            )
            nc.sync.dma_start(out=out_maskt[:, :], in_=maskt)

    nc.compile()
    return nc


_NC_CACHE = None
_PROFILE = False     # test harness may set True to route via run_bass_kernel_spmd
LAST_RESULT = None   # BassKernelResults of the most recent run


def _get_nc():
    global _NC_CACHE
    if _NC_CACHE is None:
        _NC_CACHE = _build_kernel()
    return _NC_CACHE


def _zeta_table(b3: np.ndarray) -> np.ndarray:
    """Logit-space thresholds: #{rho in [0,64): z_b >= zeta[rho]} equals
    round_half_even(clip(4 + 60*sigmoid(z_b + b3))), and since zeta is
    ascending, mask = (z >= zeta[rank]). Computed in float64."""
    b3v = float(np.asarray(b3, np.float64).reshape(-1)[0])
    zt = np.empty(MAX_STATES, np.float64)
    for rho in range(MAX_STATES):
        k = rho + 1  # threshold for reaching >= k states
        t = (k - 4.5) / (MAX_STATES - MIN_STATES)
        if t <= 0.0:
            zt[rho] = -1e30
        elif t >= 1.0:
            zt[rho] = 1e30
        else:
            zt[rho] = math.log(t / (1.0 - t)) - b3v
    return zt.astype(np.float32)


def _host_consts(inputs) -> dict:
    W1 = np.asarray(inputs["W1"], np.float32)
    b1 = np.asarray(inputs["b1"], np.float32)
    W2 = np.asarray(inputs["W2"], np.float32)
    b2 = np.asarray(inputs["b2"], np.float32)
    W3 = np.asarray(inputs["W3"], np.float32)
    b3 = np.asarray(inputs["b3"], np.float32)
    Ws1 = np.asarray(inputs["Ws1"], np.float32)
    bs1 = np.asarray(inputs["bs1"], np.float32)
    Ws2 = np.asarray(inputs["Ws2"], np.float32)
    state_bank = np.asarray(inputs["state_bank"], np.float32)
    # temperature only rescales softmax -> rank order unchanged; unused.

    pk = np.zeros((P, PK_COLS), np.float32)
    pk[:, PK_B1:PK_B1 + 4] = b1.reshape(4, P).T
    pk[:, PK_B2:PK_B2 + 2] = b2.reshape(2, P).T
    pk[:, PK_W3:PK_W3 + 2] = W3.reshape(1, 2, P)[0].T
    pk[:, PK_BS1:PK_BS1 + 2] = bs1.reshape(2, P).T
    pk[:, PK_WS2:PK_WS2 + 2] = Ws2.reshape(1, 2, P)[0].T
    pk[0:MAX_STATES, PK_IDENT:PK_IDENT + MAX_STATES] = np.eye(MAX_STATES)
    pk[0:MAX_STATES, PK_ZETA] = _zeta_table(b3)
    pk[0:MAX_STATES, PK_IOTA] = np.arange(MAX_STATES)
    pk[:, PK_RVEC] = 1.0 / S
    pk[:, PK_ONES] = 1.0
    pk[0, PK_ONESROW:PK_ONESROW + MAX_STATES] = 1.0
    for b in range(4):
        pk[:, PK_BSEL + 4 * b + b] = 1.0 / S
    pkb = np.zeros((P, PKB_COLS), mybir.dt.np(BF16))
    for b in range(B_LOC):
        pkb[:, 4 * b + b] = 1.0 / S
    pkb[:, PKB_W3:PKB_W3 + 2] = W3.reshape(1, 2, P)[0].T.astype(mybir.dt.np(BF16))
    for kc in range(2):
        pkb[:, PKB_W3REP + kc * MAX_STATES:PKB_W3REP + (kc + 1) * MAX_STATES] = (
            W3[0, kc * P:(kc + 1) * P].reshape(P, 1).astype(mybir.dt.np(BF16))
        )

    return {
        "w1t": np.ascontiguousarray(W1.T).astype(mybir.dt.np(BF16)),
        "w2t": np.ascontiguousarray(W2.T).astype(mybir.dt.np(BF16)),
        "ws1t": np.ascontiguousarray(Ws1.T),
        "sbt": np.ascontiguousarray(
            state_bank.T.reshape(4, P, MAX_STATES).transpose(1, 0, 2).reshape(P, 4 * MAX_STATES)
        ),
        "sbank": np.ascontiguousarray(state_bank),
        "pk": pk,
        "pkb": pkb,
    }


_RUNNER = None


def _make_runner(nc):
    """Build a cached jitted shard_map executor mirroring
    bass2jax.run_bass_via_pjrt (which re-traces on every call)."""
    import jax
    from jax.sharding import Mesh, PartitionSpec, NamedSharding
    try:
        from jax.experimental.shard_map import shard_map
    except ImportError:  # newer jax
        from jax import shard_map
    from concourse import bass2jax

    partition_name = nc.partition_id_tensor.name if nc.partition_id_tensor else None
    in_names, out_names, out_avals, zero_shapes = [], [], [], []
    for alloc in nc.m.functions[0].allocations:
        if not isinstance(alloc, mybir.MemoryLocationSet):
            continue
        name = alloc.memorylocations[0].name
        if alloc.kind == "ExternalInput":
            if name != partition_name:
                in_names.append(name)
        elif alloc.kind == "ExternalOutput":
            out_names.append(name)
            out_avals.append(
                jax.core.ShapedArray(
                    tuple(alloc.tensor_shape), mybir.dt.np(alloc.dtype)
                )
            )
            zero_shapes.append(
                (tuple(alloc.tensor_shape), mybir.dt.np(alloc.dtype))
            )
    n_params, n_outs = len(in_names), len(out_avals)
    names_all = tuple(
        in_names + out_names + ([partition_name] if partition_name else [])
    )

    def _body(*args):
        ops = list(args)
        if partition_name:
            ops.append(bass2jax.partition_id_tensor())
        return tuple(
            bass2jax._bass_exec_p.bind(
                *ops,
                out_avals=tuple(out_avals),
                in_names=names_all,
                out_names=tuple(out_names),
                lowering_input_output_aliases=(),
                sim_require_finite=True,
                sim_require_nnan=True,
                nc=nc,
            )
        )

    devices = jax.devices()[:N_CORES]
    mesh = Mesh(np.asarray(devices), ("core",))
    sharded = jax.jit(
        shard_map(
            _body,
            mesh=mesh,
            in_specs=(PartitionSpec("core"),) * (n_params + n_outs),
            out_specs=(PartitionSpec("core"),) * n_outs,
            check_rep=False,
        ),
        donate_argnums=tuple(range(n_params, n_params + n_outs)),
        keep_unused=True,
    )
    shardng = NamedSharding(mesh, PartitionSpec("core"))

    def run(global_ins):
        cin = [jax.device_put(global_ins[n], shardng) for n in in_names]
        zeros = [
            jax.device_put(np.zeros((N_CORES * s[0], *s[1:]), d), shardng)
            for (s, d) in zero_shapes
        ]
        outs = sharded(*cin, *zeros)
        outs = [np.asarray(o) for o in outs]
        return {
            name: outs[i].reshape(N_CORES, *out_avals[i].shape)
            for i, name in enumerate(out_names)
        }

    return run


def _prep_x(x_f32: np.ndarray) -> np.ndarray:
    """Round-to-nearest-even bf16 (identical to the DMA cast the kernel
    previously did on-device; 250x margin verified for the mask math)."""
    return np.ascontiguousarray(x_f32.astype(mybir.dt.np(BF16)))


def kernel(**inputs) -> tuple:
    x = _prep_x(np.asarray(inputs["x"], np.float32))
    consts = _host_consts(inputs)
    in_maps = [
        {"x": x[i * B_LOC:(i + 1) * B_LOC], **consts} for i in range(N_CORES)
    ]

    nc = _get_nc()
    if _PROFILE:
        res = run_bass_kernel_spmd(
            nc, in_maps, core_ids=list(range(N_CORES))
        )
        global LAST_RESULT
        LAST_RESULT = res
        results = res.results
        allocated = np.concatenate(
            [results[i]["out_states"] for i in range(N_CORES)], axis=0
        )
        mask = np.concatenate(
            [results[i]["out_maskt"].T for i in range(N_CORES)], axis=0
        ) > 0.5
        return allocated, mask

    global _RUNNER
    if _RUNNER is None:
        _RUNNER = _make_runner(nc)
    global_ins = {"x": x}
    for name, arr in consts.items():
        global_ins[name] = np.tile(arr, (N_CORES,) + (1,) * (arr.ndim - 1))
    outs = _RUNNER(global_ins)
    allocated = np.ascontiguousarray(
        outs["out_states"].reshape(B, MAX_STATES, D_ST)
    )
    mask = np.ascontiguousarray(
        outs["out_maskt"].transpose(0, 2, 1).reshape(B, MAX_STATES)
    ) > 0.5
    return allocated, mask


# revision 39
# speedup vs baseline: 1.0074x; 1.0074x over previous
"""Trainium2 Bass kernel for AdaptiveStateAllocator (topk_masking).

Data-parallel over batch on 8 NeuronCores. Each core:
  - streams its x shard [4, 2048, 1024] from HBM as host-pre-rounded bf16
    (16MB/core instead of 32MB -- the memory-bound part; rounding validated
    at ~250x margin for the mask math),
  - mean-pools over seq via PE matmuls with a stationary per-batch 1/S
    selector column (x is the moving operand, no per-matmul weight loads),
  - runs the tiny complexity MLP and importance-scorer MLP on-device,
  - derives the mask without sorting or sigmoid:
      rank_i   = #{j : logit_j > logit_i}            (comparison matrix)
      mask^T[i,b] = (z_b >= zeta[rank_i])            (z = pre-sigmoid logit)
    where zeta[rho] = logit((rho - 3.5)/60) - b3 is a host-precomputed
    threshold table equivalent to round_half_even(4 + 60*sigmoid(z + b3)).
    The zeta[rank_i] gather runs as a one-hot matmul BEFORE the x stream
    finishes, so the post-stream tail is just pooled->h1->h2->z->mask.
  - emits its [64, 4] mask^T shard and [4, 64, 512] broadcast state bank.

Host side shards inputs, precomputes transposed weights + the threshold
table, and reassembles the full outputs.
"""

import math
import sys

import numpy as np

sys.path.insert(0, "/opt/trn_rl_repo")

import concourse.bass as bass  # noqa: E402
import concourse.tile as tile  # noqa: E402
from concourse import mybir  # noqa: E402
from concourse.bacc import Bacc  # noqa: E402
from concourse.bass_utils import run_bass_kernel_spmd  # noqa: E402

F32 = mybir.dt.float32
BF16 = mybir.dt.bfloat16
AF = mybir.ActivationFunctionType
ALU = mybir.AluOpType

# x and the complexity-MLP weights run in bf16 (empirical mask-boundary
# margin 0.19 in v-space vs <=0.0011 bf16-induced error on these inputs).
# The importance-logit branch (tight 8.8e-5 rank-boundary margin) stays
# fp32 throughout.

# Problem constants (hardcoded per harness contract).
B, S, D_IN, D_ST = 32, 2048, 1024, 512
MIN_STATES, MAX_STATES = 4, 64
N_CORES = 8
B_LOC = B // N_CORES          # 4 samples per core
P = 128                       # SBUF partitions
SEQ_GROUP = 512               # seq rows per DMA tile -> [128, 4, 1024] bf16 = 1MB
N_SUB = SEQ_GROUP // P        # 4 seq sub-chunks per tile
N_TILES = S // SEQ_GROUP      # 4 tiles per sample
K_CHUNKS = S // P             # 16 accumulation chunks per sample
D_CHUNKS = D_IN // P          # 8

# packed small-const column layout ([128, PK_COLS] f32)
PK_B1 = 0          # [128, 4]  b1 chunks
PK_B2 = 4          # [128, 2]
PK_W3 = 6          # [128, 2]
PK_BS1 = 8         # [128, 2]
PK_WS2 = 10        # [128, 2]
PK_IDENT = 12      # [64, 64] identity (partitions 0-63)
PK_ZETA = 76       # [64, 1]  zeta_col
PK_IOTA = 77       # [64, 1]  0..63
PK_RVEC = 78       # [128, 1] 1/S
PK_ONES = 79       # [128, 1] ones (spare)
PK_ONESROW = 80    # [1, 64] ones in partition 0
PK_BSEL = 148      # [128, 4] x4: block b has column b = 1/S, rest 0
PK_COLS = 164
# bf16 const block: bsel blocks then W3 chunks
PKB_W3 = 16        # [128, 2]
PKB_W3REP = 18     # [128, 64] x2: W3 chunk kc replicated across 64 columns
PKB_COLS = 146


def _build_kernel(n_sweeps: int = 1):
    """n_sweeps > 1 repeats the x stream + pooling (profiling only): the
    marginal cost per extra sweep isolates true HW time from dispatch noise."""
    nc = Bacc()

    x = nc.declare_dram_parameter("x", [B_LOC, S, D_IN], BF16, isOutput=False)
    w1t = nc.declare_dram_parameter("w1t", [D_IN, D_ST], BF16, isOutput=False)
    w2t = nc.declare_dram_parameter("w2t", [D_ST, D_ST // 2], BF16, isOutput=False)
    ws1t = nc.declare_dram_parameter("ws1t", [D_ST, D_ST // 2], F32, isOutput=False)
    sbt = nc.declare_dram_parameter("sbt", [P, 4 * MAX_STATES], F32, isOutput=False)
    sbank = nc.declare_dram_parameter("sbank", [MAX_STATES, D_ST], F32, isOutput=False)
    pk = nc.declare_dram_parameter("pk", [P, PK_COLS], F32, isOutput=False)
    pkb = nc.declare_dram_parameter("pkb", [P, PKB_COLS], BF16, isOutput=False)

    out_states = nc.declare_dram_parameter(
        "out_states", [B_LOC, MAX_STATES, D_ST], F32, isOutput=True
    )
    out_maskt = nc.declare_dram_parameter(
        "out_maskt", [MAX_STATES, B_LOC], F32, isOutput=True
    )

    with tile.TileContext(nc) as tc:
        with (
            tc.tile_pool(name="xpool", bufs=8) as xpool,
            tc.tile_pool(name="consts", bufs=1) as consts,
            tc.tile_pool(name="small", bufs=1) as small,
            tc.tile_pool(name="psum", bufs=4, space="PSUM") as psum,
            tc.tile_pool(name="psum1", bufs=1, space="PSUM") as psum1,
        ):
            # ---- x stream: issue on the sync HWDGE ring, first in program
            # order so the stream starts immediately. Pooling matmuls keep the
            # 1/S column STATIONARY (one trivial weight load) and stream x as
            # the moving operand in N=512 slabs -> pooled [4, 1024] in PSUM.
            # (The stationary-x variant pays a [128,128] LDWEIGHTS per matmul
            # and throttles the stream on real HW.)
            pool_ps = psum1.tile([B_LOC, D_IN], F32)           # 2 PSUM banks
            poolt_ps = psum1.tile([P, D_CHUNKS * B_LOC], F32)  # one PSUM bank

            # x is pre-rounded to bf16 on the host, so the stream is plain
            # HWDGE loads of half the bytes (no SWDGE cast ring needed).

            # Tile plan: 2MB tiles, except the final ~2MB of the last batch
            # which is split into 512KB tiles so the post-stream matmul burst
            # (which is pure tail) stays short.
            def tile_plan():
                plan = []  # (b, seq_start, n_sub)
                for b in range(B_LOC):
                    s = 0
                    while s < S:
                        if b == B_LOC - 1 and s >= S - SEQ_GROUP:
                            step = P  # 512KB tiles for the last seq group
                        else:
                            step = SEQ_GROUP
                        plan.append((b, s, step // P))
                        s += step
                return plan

            def emit_x_dmas():
                tiles = []
                for idx, (b, s, n_sub) in enumerate(tile_plan()):
                    xt = xpool.tile([P, N_SUB, D_IN], BF16, tag="xtb")
                    nc.sync.dma_start(
                        out=xt[:, 0:n_sub, :],
                        in_=x[b, s:s + n_sub * P, :].rearrange(
                            "(n p) d -> p n d", p=P
                        ),
                    )
                    tiles.append((idx, b, s, n_sub, xt))
                return tiles

            def emit_pooling(tiles):
                for idx, b, s, n_sub, xt in tiles:
                    bsel = pkb_sb[:, 4 * b:4 * b + 4]
                    for n in range(n_sub):
                        kc = s // P + n
                        for half in range(2):
                            nc.tensor.matmul(
                                pool_ps[:, half * 512:(half + 1) * 512],
                                lhsT=bsel,
                                rhs=xt[:, n, half * 512:(half + 1) * 512],
                                start=(kc == 0 and b == 0),
                                stop=(kc == K_CHUNKS - 1 and b == B_LOC - 1),
                            )

            x_tiles = emit_x_dmas()

            # ---- const loads: scalar-engine HWDGE ring (doesn't queue
            # behind the x stream's slot-reuse waits)
            pk_sb = consts.tile([P, PK_COLS], F32)
            nc.scalar.dma_start(out=pk_sb, in_=pk[:, :])
            pkb_sb = consts.tile([P, PKB_COLS], BF16)
            nc.scalar.dma_start(out=pkb_sb, in_=pkb[:, :])
            w1_sb = consts.tile([P, D_CHUNKS, D_ST], BF16)  # [128, 8, 512]
            nc.scalar.dma_start(out=w1_sb, in_=w1t.rearrange("(c p) m -> p c m", p=P))
            w2_sb = consts.tile([P, 4, D_ST // 2], BF16)
            nc.scalar.dma_start(out=w2_sb, in_=w2t.rearrange("(c p) m -> p c m", p=P))
            ws1_sb = consts.tile([P, 4, D_ST // 2], F32)
            nc.scalar.dma_start(out=ws1_sb, in_=ws1t.rearrange("(c p) m -> p c m", p=P))
            sbt_sb = consts.tile([P, 4, MAX_STATES], F32)
            nc.scalar.dma_start(out=sbt_sb, in_=sbt.rearrange("p (c m) -> p c m", m=MAX_STATES))
            sbank_sb = consts.tile([MAX_STATES, D_ST], F32)
            nc.scalar.dma_start(out=sbank_sb, in_=sbank[:, :])

            ones_row = pk_sb[0:1, PK_ONESROW:PK_ONESROW + MAX_STATES]
            ident_sb = pk_sb[0:MAX_STATES, PK_IDENT:PK_IDENT + MAX_STATES]
            zeta_col = pk_sb[0:MAX_STATES, PK_ZETA:PK_ZETA + 1]
            iota_col = pk_sb[0:MAX_STATES, PK_IOTA:PK_IOTA + 1]

            # ---- broadcast state bank to output (tiny, independent)
            for b in range(B_LOC):
                nc.sync.dma_start(out=out_states[b], in_=sbank_sb)

            # ---- pooling matmuls, first half: PE's in-order queue must
            # start with tile consumption so DMA slots recycle promptly
            emit_pooling(x_tiles[:9])

            # ==== importance branch (independent of x -> runs during stream,
            # its cross-engine hops absorbed while tiles 8.. buffer up)
            hs_t = small.tile([P, 2 * MAX_STATES], F32)
            for mc in range(2):
                hs_ps = psum.tile([P, MAX_STATES], F32, tag="mlp")
                for kc in range(4):
                    nc.tensor.matmul(
                        hs_ps,
                        lhsT=ws1_sb[:, kc, mc * P:(mc + 1) * P],
                        rhs=sbt_sb[:, kc, :],
                        start=(kc == 0),
                        stop=(kc == 3),
                    )
                nc.scalar.activation(
                    out=hs_t[:, mc * MAX_STATES:(mc + 1) * MAX_STATES],
                    in_=hs_ps,
                    func=AF.Relu,
                    bias=pk_sb[:, PK_BS1 + mc:PK_BS1 + mc + 1],
                )
            # importance logits, column+row forms (bs2 dropped: rank-invariant)
            l_ps = psum.tile([MAX_STATES, 1], F32, tag="mlp")
            for kc in range(2):
                nc.tensor.matmul(
                    l_ps,
                    lhsT=hs_t[:, kc * MAX_STATES:(kc + 1) * MAX_STATES],
                    rhs=pk_sb[:, PK_WS2 + kc:PK_WS2 + kc + 1],
                    start=(kc == 0),
                    stop=(kc == 1),
                )
            lcol = small.tile([MAX_STATES, 1], F32)
            nc.scalar.copy(out=lcol, in_=l_ps)
            lrow_ps = psum.tile([1, MAX_STATES], F32, tag="mlp")
            nc.tensor.matmul(lrow_ps, lhsT=lcol, rhs=ident_sb, start=True, stop=True)
            lrow = small.tile([1, MAX_STATES], F32)
            nc.scalar.copy(out=lrow, in_=lrow_ps)
            # lrep[i, j] = l_j (K=1 outer product), then rank_i = #{j: l_j > l_i}
            lrep_ps = psum.tile([MAX_STATES, MAX_STATES], F32, tag="mlp")
            nc.tensor.matmul(lrep_ps, lhsT=ones_row, rhs=lrow, start=True, stop=True)
            cmp = small.tile([MAX_STATES, MAX_STATES], F32)
            rcol = small.tile([MAX_STATES, 1], F32)
            nc.vector.tensor_scalar(
                out=cmp, in0=lrep_ps, scalar1=lcol, scalar2=0.0, op0=ALU.is_gt,
                op1=ALU.add, accum_out=rcol,
            )
            # rank one-hot: OhT[rho, i] = (rank_i == rho); zeta_r = OhT.T @ zeta
            rrow_ps = psum.tile([1, MAX_STATES], F32, tag="mlp")
            nc.tensor.matmul(rrow_ps, lhsT=rcol, rhs=ident_sb, start=True, stop=True)
            rrow = small.tile([1, MAX_STATES], F32)
            nc.scalar.copy(out=rrow, in_=rrow_ps)
            rrep_ps = psum.tile([MAX_STATES, MAX_STATES], F32, tag="mlp")
            nc.tensor.matmul(rrep_ps, lhsT=ones_row, rhs=rrow, start=True, stop=True)
            oht = small.tile([MAX_STATES, MAX_STATES], F32)
            nc.vector.tensor_scalar(
                out=oht, in0=rrep_ps, scalar1=iota_col, scalar2=None, op0=ALU.is_equal
            )
            zr_ps = psum.tile([MAX_STATES, 1], F32, tag="mlp")
            nc.tensor.matmul(zr_ps, lhsT=oht, rhs=zeta_col, start=True, stop=True)
            zeta_r = small.tile([MAX_STATES, 1], F32)
            nc.scalar.copy(out=zeta_r, in_=zr_ps)

            # ---- pooling matmuls, second half (first half emitted above so
            # PE starts consuming tiles before the rank chain's engine hops)
            emit_pooling(x_tiles[9:])
            for _sweep in range(1, n_sweeps):
                emit_pooling(emit_x_dmas())

            # ==== complexity MLP (critical tail after the x stream)
            # PSUM->SBUF eviction split across ACT and DVE halves (concurrent)
            pooled_sb = small.tile([B_LOC, D_IN], F32)
            nc.scalar.copy(
                out=pooled_sb[:, 0:D_IN // 2], in_=pool_ps[:, 0:D_IN // 2]
            )
            nc.vector.tensor_copy(
                out=pooled_sb[:, D_IN // 2:], in_=pool_ps[:, D_IN // 2:]
            )
            for dc in range(D_CHUNKS):
                nc.tensor.matmul(
                    poolt_ps[:, dc * B_LOC:(dc + 1) * B_LOC],
                    lhsT=pooled_sb[0:B_LOC, dc * P:(dc + 1) * P],
                    rhs=ident_sb[0:B_LOC, 0:B_LOC],
                    start=True,
                    stop=True,
                )
            pooled_t = small.tile([P, D_CHUNKS * B_LOC], BF16)  # [128, 32]
            nc.scalar.copy(out=pooled_t, in_=poolt_ps)
            h1_t = small.tile([P, 4 * B_LOC], BF16)
            for mc in range(4):
                h1_ps = psum.tile([P, B_LOC], F32, tag="mlp")
                for kc in range(D_CHUNKS):
                    nc.tensor.matmul(
                        h1_ps,
                        lhsT=w1_sb[:, kc, mc * P:(mc + 1) * P],
                        rhs=pooled_t[:, kc * B_LOC:(kc + 1) * B_LOC],
                        start=(kc == 0),
                        stop=(kc == D_CHUNKS - 1),
                    )
                nc.scalar.activation(
                    out=h1_t[:, mc * B_LOC:(mc + 1) * B_LOC],
                    in_=h1_ps,
                    func=AF.Relu,
                    bias=pk_sb[:, PK_B1 + mc:PK_B1 + mc + 1],
                )
            h2_t = small.tile([P, 2 * B_LOC], BF16)
            for mc in range(2):
                h2_ps = psum.tile([P, B_LOC], F32, tag="mlp")
                for kc in range(4):
                    nc.tensor.matmul(
                        h2_ps,
                        lhsT=w2_sb[:, kc, mc * P:(mc + 1) * P],
                        rhs=h1_t[:, kc * B_LOC:(kc + 1) * B_LOC],
                        start=(kc == 0),
                        stop=(kc == 3),
                    )
                nc.scalar.activation(
                    out=h2_t[:, mc * B_LOC:(mc + 1) * B_LOC],
                    in_=h2_ps,
                    func=AF.Relu,
                    bias=pk_sb[:, PK_B2 + mc:PK_B2 + mc + 1],
                )
            # z replicated [64, 4] in ONE accumulation: lhsT = W3 chunk
            # broadcast across 64 columns (host-prepared), rhs = h2^T
            zrep_ps = psum.tile([MAX_STATES, B_LOC], F32, tag="mlp")
            for kc in range(2):
                nc.tensor.matmul(
                    zrep_ps,
                    lhsT=pkb_sb[:, PKB_W3REP + kc * MAX_STATES:
                                PKB_W3REP + (kc + 1) * MAX_STATES],
                    rhs=h2_t[:, kc * B_LOC:(kc + 1) * B_LOC],
                    start=(kc == 0),
                    stop=(kc == 1),
                )
            # mask^T[i, b] = (z_b >= zeta[rank_i])
            maskt = small.tile([MAX_STATES, B_LOC], F32)
            nc.vector.tensor_scalar(
                out=maskt, in0=zrep_ps, scalar1=zeta_r, scalar2=None, op0=ALU.is_ge
            )
            nc.sync.dma_start(out=out_maskt[:, :], in_=maskt)

    nc.compile()
    return nc


_NC_CACHE = None
_PROFILE = False     # test harness may set True to route via run_bass_kernel_spmd
LAST_RESULT = None   # BassKernelResults of the most recent run


def _get_nc():
    global _NC_CACHE
    if _NC_CACHE is None:
        _NC_CACHE = _build_kernel()
    return _NC_CACHE


def _zeta_table(b3: np.ndarray) -> np.ndarray:
    """Logit-space thresholds: #{rho in [0,64): z_b >= zeta[rho]} equals
    round_half_even(clip(4 + 60*sigmoid(z_b + b3))), and since zeta is
    ascending, mask = (z >= zeta[rank]). Computed in float64."""
    b3v = float(np.asarray(b3, np.float64).reshape(-1)[0])
    zt = np.empty(MAX_STATES, np.float64)
    for rho in range(MAX_STATES):
        k = rho + 1  # threshold for reaching >= k states
        t = (k - 4.5) / (MAX_STATES - MIN_STATES)
        if t <= 0.0:
            zt[rho] = -1e30
        elif t >= 1.0:
            zt[rho] = 1e30
        else:
            zt[rho] = math.log(t / (1.0 - t)) - b3v
    return zt.astype(np.float32)


def _host_consts(inputs) -> dict:
    W1 = np.asarray(inputs["W1"], np.float32)
    b1 = np.asarray(inputs["b1"], np.float32)
    W2 = np.asarray(inputs["W2"], np.float32)
    b2 = np.asarray(inputs["b2"], np.float32)
    W3 = np.asarray(inputs["W3"], np.float32)
    b3 = np.asarray(inputs["b3"], np.float32)
    Ws1 = np.asarray(inputs["Ws1"], np.float32)
    bs1 = np.asarray(inputs["bs1"], np.float32)
    Ws2 = np.asarray(inputs["Ws2"], np.float32)
    state_bank = np.asarray(inputs["state_bank"], np.float32)
    # temperature only rescales softmax -> rank order unchanged; unused.

    pk = np.zeros((P, PK_COLS), np.float32)
    pk[:, PK_B1:PK_B1 + 4] = b1.reshape(4, P).T
    pk[:, PK_B2:PK_B2 + 2] = b2.reshape(2, P).T
    pk[:, PK_W3:PK_W3 + 2] = W3.reshape(1, 2, P)[0].T
    pk[:, PK_BS1:PK_BS1 + 2] = bs1.reshape(2, P).T
    pk[:, PK_WS2:PK_WS2 + 2] = Ws2.reshape(1, 2, P)[0].T
    pk[0:MAX_STATES, PK_IDENT:PK_IDENT + MAX_STATES] = np.eye(MAX_STATES)
    pk[0:MAX_STATES, PK_ZETA] = _zeta_table(b3)
    pk[0:MAX_STATES, PK_IOTA] = np.arange(MAX_STATES)
    pk[:, PK_RVEC] = 1.0 / S
    pk[:, PK_ONES] = 1.0
    pk[0, PK_ONESROW:PK_ONESROW + MAX_STATES] = 1.0
    for b in range(4):
        pk[:, PK_BSEL + 4 * b + b] = 1.0 / S
    pkb = np.zeros((P, PKB_COLS), mybir.dt.np(BF16))
    for b in range(B_LOC):
        pkb[:, 4 * b + b] = 1.0 / S
    pkb[:, PKB_W3:PKB_W3 + 2] = W3.reshape(1, 2, P)[0].T.astype(mybir.dt.np(BF16))
    for kc in range(2):
        pkb[:, PKB_W3REP + kc * MAX_STATES:PKB_W3REP + (kc + 1) * MAX_STATES] = (
            W3[0, kc * P:(kc + 1) * P].reshape(P, 1).astype(mybir.dt.np(BF16))
        )

    return {
        "w1t": np.ascontiguousarray(W1.T).astype(mybir.dt.np(BF16)),
        "w2t": np.ascontiguousarray(W2.T).astype(mybir.dt.np(BF16)),
        "ws1t": np.ascontiguousarray(Ws1.T),
        "sbt": np.ascontiguousarray(
            state_bank.T.reshape(4, P, MAX_STATES).transpose(1, 0, 2).reshape(P, 4 * MAX_STATES)
        ),
        "sbank": np.ascontiguousarray(state_bank),
        "pk": pk,
        "pkb": pkb,
    }


_RUNNER = None


def _make_runner(nc):
    """Build a cached jitted shard_map executor mirroring
    bass2jax.run_bass_via_pjrt (which re-traces on every call)."""
    import jax
    from jax.sharding import Mesh, PartitionSpec, NamedSharding
    try:
        from jax.experimental.shard_map import shard_map
    except ImportError:  # newer jax
        from jax import shard_map
    from concourse import bass2jax

    partition_name = nc.partition_id_tensor.name if nc.partition_id_tensor else None
    in_names, out_names, out_avals, zero_shapes = [], [], [], []
    for alloc in nc.m.functions[0].allocations:
        if not isinstance(alloc, mybir.MemoryLocationSet):
            continue
        name = alloc.memorylocations[0].name
        if alloc.kind == "ExternalInput":
            if name != partition_name:
                in_names.append(name)
        elif alloc.kind == "ExternalOutput":
            out_names.append(name)
            out_avals.append(
                jax.core.ShapedArray(
                    tuple(alloc.tensor_shape), mybir.dt.np(alloc.dtype)
                )
            )
            zero_shapes.append(
                (tuple(alloc.tensor_shape), mybir.dt.np(alloc.dtype))
            )
    n_params, n_outs = len(in_names), len(out_avals)
    names_all = tuple(
        in_names + out_names + ([partition_name] if partition_name else [])
    )

    def _body(*args):
        ops = list(args)
        if partition_name:
            ops.append(bass2jax.partition_id_tensor())
        return tuple(
            bass2jax._bass_exec_p.bind(
                *ops,
                out_avals=tuple(out_avals),
                in_names=names_all,
                out_names=tuple(out_names),
                lowering_input_output_aliases=(),
                sim_require_finite=True,
                sim_require_nnan=True,
                nc=nc,
            )
        )

    devices = jax.devices()[:N_CORES]
    mesh = Mesh(np.asarray(devices), ("core",))
    sharded = jax.jit(
        shard_map(
            _body,
            mesh=mesh,
            in_specs=(PartitionSpec("core"),) * (n_params + n_outs),
            out_specs=(PartitionSpec("core"),) * n_outs,
            check_rep=False,
        ),
        donate_argnums=tuple(range(n_params, n_params + n_outs)),
        keep_unused=True,
    )
    shardng = NamedSharding(mesh, PartitionSpec("core"))

    def run(global_ins):
        cin = [jax.device_put(global_ins[n], shardng) for n in in_names]
        zeros = [
            jax.device_put(np.zeros((N_CORES * s[0], *s[1:]), d), shardng)
            for (s, d) in zero_shapes
        ]
        outs = sharded(*cin, *zeros)
        outs = [np.asarray(o) for o in outs]
        return {
            name: outs[i].reshape(N_CORES, *out_avals[i].shape)
            for i, name in enumerate(out_names)
        }

    return run


def _prep_x(x_f32: np.ndarray) -> np.ndarray:
    """Round-to-nearest-even bf16 (identical to the DMA cast the kernel
    previously did on-device; 250x margin verified for the mask math)."""
    return np.ascontiguousarray(x_f32.astype(mybir.dt.np(BF16)))


def kernel(**inputs) -> tuple:
    x = _prep_x(np.asarray(inputs["x"], np.float32))
    consts = _host_consts(inputs)
    in_maps = [
        {"x": x[i * B_LOC:(i + 1) * B_LOC], **consts} for i in range(N_CORES)
    ]

    nc = _get_nc()
    if _PROFILE:
        res = run_bass_kernel_spmd(
            nc, in_maps, core_ids=list(range(N_CORES))
        )
        global LAST_RESULT
        LAST_RESULT = res
        results = res.results
        allocated = np.concatenate(
            [results[i]["out_states"] for i in range(N_CORES)], axis=0
        )
        mask = np.concatenate(
            [results[i]["out_maskt"].T for i in range(N_CORES)], axis=0
        ) > 0.5
        return allocated, mask

    global _RUNNER
    if _RUNNER is None:
        _RUNNER = _make_runner(nc)
    global_ins = {"x": x}
    for name, arr in consts.items():
        global_ins[name] = np.tile(arr, (N_CORES,) + (1,) * (arr.ndim - 1))
    outs = _RUNNER(global_ins)
    allocated = np.ascontiguousarray(
        outs["out_states"].reshape(B, MAX_STATES, D_ST)
    )
    mask = np.ascontiguousarray(
        outs["out_maskt"].transpose(0, 2, 1).reshape(B, MAX_STATES)
    ) > 0.5
    return allocated, mask
